# revision 2
# baseline (speedup 1.0000x reference)
"""Blockwise reconditioner (block-16 normalization) on 8 Trainium2 cores.

Math per row r, block g (block size 16):
    mean = mean(x[r, 16g:16g+16])
    var  = sum((x - mean)^2) / 15          (unbiased, ddof=1)
    out  = (x - mean) / sqrt(var + 1e-5) * scales[g] + shifts[g]

Implemented as out = x * a + b with per-block coefficients
    a = scales[g] / sqrt(var + eps)
    b = shifts[g] - mean * a
using raw = sum(x^2) - s1^2/16, var = raw/15 (s1 = block sum).

bf16 data path (tolerance 2e-2; measures ~3.4e-3 end-to-end): host casts
x fp32 -> bf16, device computes in bf16/fp32, host casts back.

V3 rebalance (measured engine rates, see session notes):
  - DVE TT dense pass [128,4096] = 2.29us (no bf16 2x mode on TT);
    DVE PSUM->SBUF copy [128,2048] = 1.33us; DVE TR = 4.9us/chunk.
  - ACT pass [128,2048] = 2.2us; small [128,256] ops ~0.5us.
  - GpSimd dense TT = ~9.3us/pass (4x slower than DVE); tree = ~10us.
  - PE (LDWEIGHTS+MATMUL) pair ~73ns mixed.
Assignment per chunk (8 chunks of [128, 4096] per core):
  - squares: ACT (only cheap PSUM->SBUF squarer)
  - s1 route: chunk 0 = DVE tensor_reduce (hidden in DMA fill shadow);
    chunks 1,3,4,5,6 = DVE copy of xT PSUM->SBUF + PE mask-matmuls;
    chunks 2,7 = GpSimd adder tree (no PE/DVE cost)
  - rstd = ACT Abs_reciprocal_sqrt(raw/15 + eps)  (off the DVE; the
    Rsqrt table entry is blocked but Abs_reciprocal_sqrt measures
    bf16-exact on HW)
  - apply: mul on DVE; add on DVE except chunks 4,5 -> GpSimd
  - psA transpose tiles are [128, 2048] (2 PSUM banks): 1 ACT square +
    1 DVE copy per half-chunk instead of 2+2.
"""

import sys

import numpy as np
import ml_dtypes

for _p in ("/opt/trn_rl_repo",):
    if _p not in sys.path:
        sys.path.insert(0, _p)

import concourse.bacc as bacc
import concourse.bass as bass
import concourse.tile as tile
from concourse import mybir
from concourse.bass_utils import run_bass_kernel_spmd

F32 = mybir.dt.float32
BF16 = mybir.dt.bfloat16
ALU = mybir.AluOpType
AFT = mybir.ActivationFunctionType

N_CORES = 8
B_FULL = 4096          # total rows
N = 8192               # features
BLOCK = 16
NB = N // BLOCK        # 512 blocks
EPS = 1e-5
R = B_FULL // N_CORES  # 512 rows per core

CW = 4096              # column chunk width

# per-chunk-index strategy tables (8 chunks)
S1_ROUTE = ["dve", "copy", "tree", "copy", "copy", "copy", "copy", "tree"]
ADD_GPS = [False, False, False, False, True, True, False, False]


def build_nc(rows: int = R, cols: int = N, cw: int = CW) -> bass.Bass:
    nrt = rows // 128           # 4 row tiles
    ncc = cols // cw            # chunks per row tile
    mspc = 16                   # sub-blocks per mask accumulation group

    nc = bacc.Bacc("TRN2", target_bir_lowering=False, debug=False,
                   num_devices=N_CORES)
    x = nc.declare_dram_parameter("x", [rows, cols], BF16, isOutput=False)
    scd = nc.declare_dram_parameter("scd", [2 * NB], BF16, isOutput=False)
    shd = nc.declare_dram_parameter("shd", [2 * NB], BF16, isOutput=False)
    identbf = nc.declare_dram_parameter("identbf", [128, 128], BF16,
                                        isOutput=False)
    # maskall[f, k*128 + g] = 1 iff g == 8k + f//16 (16 sub-blocks per
    # 2048-col accumulation group)
    mask = nc.declare_dram_parameter("maskall", [128, mspc * 128], BF16,
                                     isOutput=False)
    out = nc.declare_dram_parameter("out", [rows, cols], BF16, isOutput=True)

    with tile.TileContext(nc) as tc:
        with (
            tc.tile_pool(name="singles", bufs=1) as singles,
            tc.tile_pool(name="xp", bufs=4) as xp,
            tc.tile_pool(name="sqp", bufs=2) as sqp,
            tc.tile_pool(name="trp", bufs=2) as trp,
            tc.tile_pool(name="mst", bufs=2) as mst,
            tc.tile_pool(name="cof", bufs=2) as cof,
            tc.tile_pool(name="psA", bufs=2, space="PSUM") as psA,
            tc.tile_pool(name="psB", bufs=2, space="PSUM") as psB,
        ):
            # constants: ident/mask first (first transposes block on them);
            # the scd/shd partition-broadcasts are slow 128-descriptor DMAs
            # deferred to after chunk 0's x stream.
            ident_sb = singles.tile([128, 128], BF16)
            mask_sb = singles.tile([128, mspc * 128], BF16)
            nc.gpsimd.dma_start(out=ident_sb[:, :], in_=identbf[:, :])
            nc.gpsimd.dma_start(out=mask_sb[:, :], in_=mask[:, :])
            scd_sb = singles.tile([128, 2 * NB], BF16)
            shd_sb = singles.tile([128, 2 * NB], BF16)
            eps_t = singles.tile([128, 1], F32)
            nc.vector.memset(eps_t[:, :], EPS)

            def load_coeff_consts():
                nc.sync.dma_start(out=scd_sb[:, :],
                                  in_=scd[:].partition_broadcast(128))
                nc.sync.dma_start(out=shd_sb[:, :],
                                  in_=shd[:].partition_broadcast(128))

            xts: dict = {}

            def stage_head(rt: int, c0: int, colw: int, s1_route: str,
                           slice_dma: bool = False) -> dict:
                """DMA in + transposes + squares + s1 stats source."""
                r0 = rt * 128
                sl = slice(c0, c0 + colw)
                if c0 == 0:
                    xts[rt] = xp.tile([128, cols], BF16, tag="x",
                                      name=f"xt{rt}")
                xt = xts[rt]
                if slice_dma:
                    for q in range(colw // 1024):
                        qs = slice(c0 + q * 1024, c0 + (q + 1) * 1024)
                        nc.sync.dma_start(out=xt[:, qs],
                                          in_=x[r0 : r0 + 128, qs])
                else:
                    nc.sync.dma_start(out=xt[:, sl],
                                      in_=x[r0 : r0 + 128, sl])

                nbw_c = colw // BLOCK
                uid = f"{rt}_{c0}"
                x3 = xt[:, sl].rearrange("p (g b) -> p g b", b=BLOCK)
                m_c = None
                xTs = None
                if s1_route == "dve":
                    # chunk 0 warmup: DVE is idle during the DMA fill, so
                    # block sums run there, sliced per 1024 cols to start
                    # as soon as each DMA slice lands
                    m_c = mst.tile([128, nbw_c], F32, tag="m",
                                   name=f"m_{uid}")
                    for q in range(colw // 1024):
                        qb = slice(q * 64, (q + 1) * 64)
                        nc.vector.tensor_reduce(
                            out=m_c[:, qb], in_=x3[:, qb, :],
                            op=ALU.add, axis=mybir.AxisListType.X,
                        )
                elif s1_route == "tree":
                    m_c = mst.tile([128, nbw_c], F32, tag="m",
                                   name=f"m_{uid}")
                    p1 = trp.tile([128, colw // 2], BF16, tag="p1",
                                  name=f"p1_{uid}")
                    p2 = trp.tile([128, colw // 4], BF16, tag="p2",
                                  name=f"p2_{uid}")
                    p3 = trp.tile([128, colw // 8], BF16, tag="p3",
                                  name=f"p3_{uid}")
                    nc.gpsimd.tensor_add(out=p1[:, :], in0=x3[:, :, 0:8],
                                         in1=x3[:, :, 8:16])
                    v1 = p1[:, :].rearrange("p (g b) -> p g b", b=8)
                    nc.gpsimd.tensor_add(out=p2[:, :], in0=v1[:, :, 0:4],
                                         in1=v1[:, :, 4:8])
                    v2 = p2[:, :].rearrange("p (g b) -> p g b", b=4)
                    nc.gpsimd.tensor_add(out=p3[:, :], in0=v2[:, :, 0:2],
                                         in1=v2[:, :, 2:4])
                    v3 = p3[:, :].rearrange("p (g b) -> p g b", b=2)
                    nc.gpsimd.tensor_add(out=m_c[:, :], in0=v3[:, :, 0:1],
                                         in1=v3[:, :, 1:2])
                else:
                    xTs = sqp.tile([128, colw], BF16, tag="xTs",
                                   name=f"xTs_{uid}")

                sqT = sqp.tile([128, colw], BF16, tag="sqT",
                               name=f"sqT_{uid}")
                for half in range(colw // 2048):
                    xT = psA.tile([128, 2048], BF16, tag="xT",
                                  name=f"xT_{uid}_{half}")
                    for j in range(16):
                        cj = c0 + half * 2048 + j * 128
                        nc.tensor.transpose(
                            xT[:, j * 128 : (j + 1) * 128],
                            xt[:, cj : cj + 128],
                            ident_sb[:, :],
                        )
                    hs = slice(half * 2048, (half + 1) * 2048)
                    nc.scalar.square(out=sqT[:, hs], in_=xT[:, :])
                    if s1_route == "copy":
                        nc.vector.tensor_copy(out=xTs[:, hs], in_=xT[:, :])
                return {"rt": rt, "c0": c0, "colw": colw, "xt": xt,
                        "sqT": sqT, "xTs": xTs, "m_c": m_c}

            def stage_tail(st: dict, add_gps: bool = False) -> None:
                """PE masked matmuls + coeff + apply + DMA out (1 chunk
                behind stage_head so the engine streams stay separated)."""
                rt, c0, colw, xt, sqT, xTs, m_c = (
                    st["rt"], st["c0"], st["colw"], st["xt"],
                    st["sqT"], st["xTs"], st["m_c"])
                r0 = rt * 128
                nbw_c = colw // BLOCK
                spc_c = colw // 128
                uid = f"{rt}_{c0}"

                s2_ps = psB.tile([128, nbw_c], F32, tag="s2",
                                 name=f"s2_{uid}")
                gw = min(128, nbw_c)   # block-group (and moving) width
                for k in range(spc_c):
                    grp, mk = k // 16, k % 16
                    g0 = grp * 128
                    nc.tensor.matmul(
                        s2_ps[:, g0 : g0 + gw],
                        sqT[:, k * 128 : (k + 1) * 128],
                        mask_sb[:, mk * 128 : mk * 128 + gw],
                        start=(mk == 0),
                        stop=(mk == 15 or k == spc_c - 1),
                    )
                if xTs is not None:
                    s1_ps = psB.tile([128, nbw_c], F32, tag="s1",
                                     name=f"s1_{uid}")
                    for k in range(spc_c):
                        grp, mk = k // 16, k % 16
                        g0 = grp * 128
                        nc.tensor.matmul(
                            s1_ps[:, g0 : g0 + gw],
                            xTs[:, k * 128 : (k + 1) * 128],
                            mask_sb[:, mk * 128 : mk * 128 + gw],
                            start=(mk == 0),
                            stop=(mk == 15 or k == spc_c - 1),
                        )
                    s1_src = s1_ps
                else:
                    s1_src = m_c

                mm = cof.tile([128, nbw_c], F32, tag="mm", name=f"mm_{uid}")
                raw = cof.tile([128, nbw_c], F32, tag="raw",
                               name=f"raw_{uid}")
                rstd = cof.tile([128, nbw_c], BF16, tag="rstd",
                                name=f"rstd_{uid}")
                t_dup = cof.tile([128, 2 * nbw_c], BF16, tag="td",
                                 name=f"td_{uid}")
                amr_acc = cof.tile([128, 1], F32, tag="acc",
                                   name=f"acc_{uid}")
                a_dup = cof.tile([128, 2 * nbw_c], BF16, tag="ad",
                                 name=f"ad_{uid}")
                b_dup = cof.tile([128, 2 * nbw_c], BF16, tag="bd",
                                 name=f"bd_{uid}")

                # mm = s1^2 (ACT); raw = s2 - mm/16 (DVE STT);
                # rstd = 1/sqrt(raw/15 + eps) (ACT Abs_reciprocal_sqrt)
                nc.scalar.square(out=mm[:, :], in_=s1_src[:, :])
                nc.vector.scalar_tensor_tensor(
                    out=raw[:, :], in0=mm[:, :], scalar=-1.0 / BLOCK,
                    in1=s2_ps[:, :], op0=ALU.mult, op1=ALU.add,
                )
                nc.scalar.activation(
                    out=rstd[:, :], in_=raw[:, :],
                    func=AFT.Abs_reciprocal_sqrt,
                    bias=eps_t[:, :], scale=1.0 / (BLOCK - 1),
                )
                # a_dup = scd * rstd (dup'd bf16, broadcast in1)
                nc.vector.tensor_mul(
                    out=a_dup[:, :].rearrange("p (g e) -> p g e", e=2),
                    in0=(scd_sb[:, c0 // 8 : c0 // 8 + 2 * nbw_c]
                         .rearrange("p (g e) -> p g e", e=2)),
                    in1=rstd[:, :].unsqueeze(2).broadcast_to(
                        (128, nbw_c, 2)),
                )
                # t_dup = (s1 * -1/16) * a  (dup'd, bf16) in one custom op,
                # then b = shifts + t_dup
                nc.vector.affine_mul_reduce(
                    out=t_dup[:, :].rearrange("p (g e) -> p g e", e=2),
                    accum_out=amr_acc[:, :],
                    in0=s1_src[:, :].unsqueeze(2)
                        .broadcast_to((128, nbw_c, 2)),
                    in1=a_dup[:, :].rearrange("p (g e) -> p g e", e=2),
                    scale=-1.0 / BLOCK, bias=0.0,
                )
                nc.vector.tensor_add(
                    out=b_dup[:, :], in0=t_dup[:, :],
                    in1=shd_sb[:, c0 // 8 : c0 // 8 + 2 * nbw_c],
                )

                # apply: out = x*a + b in place, then DMA out
                sl = slice(c0, c0 + colw)
                x4 = xt[:, sl].rearrange("p (g b8 e) -> p g b8 e",
                                         b8=8, e=2)
                a4 = (a_dup[:, :].rearrange("p (g e) -> p g e", e=2)
                      .unsqueeze(2).broadcast_to((128, nbw_c, 8, 2)))
                b4 = (b_dup[:, :].rearrange("p (g e) -> p g e", e=2)
                      .unsqueeze(2).broadcast_to((128, nbw_c, 8, 2)))
                nc.vector.tensor_mul(out=x4, in0=x4, in1=a4)
                add_eng = nc.gpsimd if add_gps else nc.vector
                add_eng.tensor_add(out=x4, in0=x4, in1=b4)
                nc.sync.dma_start(out=out[r0 : r0 + 128, sl],
                                  in_=xt[:, sl])

            # chunk schedule: 1-chunk lag between head and tail keeps the
            # head-stage streams (DMA/transpose/square/copy) time-shifted
            # from the tail-stage streams (matmul-read/coeff/apply).
            # Chunk 0's tail is emitted immediately so the DVE doesn't
            # idle through the fill.
            chunks = []
            for rt in range(nrt):
                cc0 = 0
                for _ in range(ncc):
                    chunks.append((rt, cc0, cw))
                    cc0 += cw
            prev = None
            prev_i = -1
            for i, (rt, cc0, w) in enumerate(chunks):
                st = stage_head(rt, cc0, w, S1_ROUTE[i], slice_dma=(i == 0))
                if i == 0:
                    load_coeff_consts()
                    stage_tail(st, add_gps=ADD_GPS[0])
                else:
                    if prev is not None:
                        stage_tail(prev, add_gps=ADD_GPS[prev_i])
                    prev = st
                    prev_i = i
            if prev is not None:
                stage_tail(prev, add_gps=ADD_GPS[prev_i])
    nc.compile()
    return nc


def aux_inputs() -> dict:
    """Constant tensors fed alongside the real inputs."""
    mspc = 16
    maskall = np.zeros((128, mspc * 128), np.float32)
    for k in range(mspc):
        for f in range(128):
            maskall[f, k * 128 + 8 * k + f // BLOCK] = 1.0
    return {
        "identbf": np.eye(128, dtype=np.float32).astype(ml_dtypes.bfloat16),
        "maskall": maskall.astype(ml_dtypes.bfloat16),
    }


_NC_CACHE: dict = {}


def _get_nc() -> bass.Bass:
    if "nc" not in _NC_CACHE:
        _NC_CACHE["nc"] = build_nc()
    return _NC_CACHE["nc"]


def run_sharded(x, scales, shifts, trace: bool = False):
    """Run the SPMD kernel on 8 cores. Returns (out, BassKernelResults)."""
    x = np.ascontiguousarray(np.asarray(x, dtype=np.float32))
    scales = np.ascontiguousarray(np.asarray(scales, dtype=np.float32))
    shifts = np.ascontiguousarray(np.asarray(shifts, dtype=np.float32))
    assert x.shape == (B_FULL, N), x.shape
    xb = x.astype(ml_dtypes.bfloat16)
    scd = np.repeat(scales, 2).astype(ml_dtypes.bfloat16)
    shd = np.repeat(shifts, 2).astype(ml_dtypes.bfloat16)
    nc = _get_nc()
    aux = aux_inputs()
    in_maps = [
        {"x": xb[i * R : (i + 1) * R], "scd": scd, "shd": shd, **aux}
        for i in range(N_CORES)
    ]
    res = run_bass_kernel_spmd(nc, in_maps, core_ids=list(range(N_CORES)),
                               trace=trace)
    outs = [np.asarray(m["out"]).astype(np.float32) for m in res.results]
    return np.concatenate(outs, axis=0), res


def kernel(x, scales, shifts):
    out, _ = run_sharded(x, scales, shifts, trace=False)
    return out
